# revision 1
# baseline (speedup 1.0000x reference)
"""Trainium2 Bass kernel for nn_NeuralCF (2-layer RGCN + NeuralCF head).

Strategy (8 NeuronCores, SPMD):
  - Shard by DESTINATION node: core c owns nodes [c*6250, (c+1)*6250).
  - One compiled device program = "RGCN layer slice":
      gather x[src] rows (indirect DMA, 128 rows/inst),
      scatter-add by dst via weighted one-hot matmul into PSUM,
      then dense per-relation GEMM  sum_r A_r @ W_r  (root term folded in as
      self-edges with w=1), output transposed slice [128, 6272].
  - Invoked twice (layer 1 on emb, layer 2 on h1). Host applies the cheap
    elementwise glue between launches (bias, relu, layernorm) and the small
    MLP head at the end.
"""
import numpy as np

import concourse.bacc as bacc
import concourse.bass as bass
import concourse.mybir as mybir
import concourse.tile as tile
from concourse.bass_utils import run_bass_kernel_spmd

# Problem constants (hardcoded per spec)
N = 50000
E = 1600000
D = 128
R = 2
B = 16384
EPS_LN = 1e-5
EPS_NORM = 1e-12

N_CORES = 8
NODES_PER_CORE = 6250
NTILES = 49            # ceil(6250/128)
SLOTS = NTILES * 128   # 6272 padded nodes per core
P = 128

_compiled = {}


def _build_program(k0, k1):
    """Build the RGCN-layer-slice SPMD program. k0/k1 = chunks per tile for
    relations 0/1 (relation 2 = self-edges, exactly 1 chunk per tile)."""
    ks = [k0, k1, 1]
    nch = NTILES * (k0 + k1 + 1)

    nc = bacc.Bacc("TRN2", target_bir_lowering=False, debug=False,
                   num_devices=N_CORES)
    table = nc.dram_tensor("table", [N, D], mybir.dt.float32, kind="ExternalInput")
    offs = nc.dram_tensor("offs", [P, nch], mybir.dt.int32, kind="ExternalInput")
    dstloc = nc.dram_tensor("dstloc", [P, nch], mybir.dt.float32, kind="ExternalInput")
    wcol = nc.dram_tensor("wcol", [P, nch], mybir.dt.float32, kind="ExternalInput")
    iota = nc.dram_tensor("iota", [P, P], mybir.dt.float32, kind="ExternalInput")
    wmat = nc.dram_tensor("wmat", [P, 3 * P], mybir.dt.float32, kind="ExternalInput")
    out = nc.dram_tensor("out", [P, SLOTS], mybir.dt.float32, kind="ExternalOutput")

    with tile.TileContext(nc) as tc:
        with (
            tc.tile_pool(name="const", bufs=1) as cpool,
            tc.tile_pool(name="atp", bufs=1) as apool,
            tc.tile_pool(name="aux", bufs=3) as auxpool,
            tc.tile_pool(name="xs", bufs=6) as xspool,
            tc.tile_pool(name="oh", bufs=6) as ohpool,
            tc.tile_pool(name="ps", bufs=4, space="PSUM") as pspool,
            tc.tile_pool(name="ps2", bufs=2, space="PSUM") as ps2pool,
            tc.tile_pool(name="outT", bufs=1) as outpool,
        ):
            iota_s = cpool.tile([P, P], mybir.dt.float32)
            nc.sync.dma_start(iota_s[:], iota[:, :])
            w_s = cpool.tile([P, 3 * P], mybir.dt.float32)
            nc.sync.dma_start(w_s[:], wmat[:, :])

            a_t = apool.tile([P, 3 * SLOTS], mybir.dt.float32)  # A^T accumulator

            for r in range(3):
                kr = ks[r]
                base = NTILES * sum(ks[:r])
                for t in range(NTILES):
                    otile = auxpool.tile([P, kr], mybir.dt.int32, tag="off")
                    dtile = auxpool.tile([P, kr], mybir.dt.float32, tag="dst")
                    wtile = auxpool.tile([P, kr], mybir.dt.float32, tag="w")
                    c0 = base + t * kr
                    nc.sync.dma_start(otile[:], offs[:, c0:c0 + kr])
                    nc.sync.dma_start(dtile[:], dstloc[:, c0:c0 + kr])
                    nc.sync.dma_start(wtile[:], wcol[:, c0:c0 + kr])
                    psum = pspool.tile([P, P], mybir.dt.float32, space="PSUM")
                    for j in range(kr):
                        xs = xspool.tile([P, P], mybir.dt.float32)
                        nc.gpsimd.indirect_dma_start(
                            out=xs[:], out_offset=None, in_=table[:, :],
                            in_offset=bass.IndirectOffsetOnAxis(
                                ap=otile[:, j:j + 1], axis=0))
                        oh = ohpool.tile([P, P], mybir.dt.float32)
                        nc.vector.tensor_scalar(
                            out=oh[:], in0=iota_s[:],
                            scalar1=dtile[:, j:j + 1], scalar2=wtile[:, j:j + 1],
                            op0=mybir.AluOpType.is_equal, op1=mybir.AluOpType.mult)
                        nc.tensor.matmul(psum[:], lhsT=xs[:], rhs=oh[:],
                                         start=(j == 0), stop=(j == kr - 1))
                    col = (r * NTILES + t) * P
                    nc.scalar.copy(out=a_t[:, col:col + P], in_=psum[:])

            out_t = outpool.tile([P, SLOTS], mybir.dt.float32)
            for t in range(NTILES):
                psum2 = ps2pool.tile([P, P], mybir.dt.float32, space="PSUM")
                for r in range(3):
                    col = (r * NTILES + t) * P
                    nc.tensor.matmul(psum2[:], lhsT=w_s[:, r * P:(r + 1) * P],
                                     rhs=a_t[:, col:col + P],
                                     start=(r == 0), stop=(r == 2))
                nc.scalar.copy(out=out_t[:, t * P:(t + 1) * P], in_=psum2[:])
            nc.sync.dma_start(out[:, :], out_t[:])

    nc.compile()
    return nc


def _prep_edges(edge_index, edge_type, edge_weight):
    """Host preprocessing: build per-core per-chunk arrays.

    Returns (k0, k1, offs[8,128,nch], dstloc[8,128,nch], wcol[8,128,nch]).
    Chunk column layout: r-major, then tile, then chunk-within-tile.
    """
    src = edge_index[0].astype(np.int64)
    dst = edge_index[1].astype(np.int64)
    et = edge_type.astype(np.int64)
    w = edge_weight.astype(np.float32)

    core = dst // NODES_PER_CORE
    pos = dst % NODES_PER_CORE
    tl = pos // P
    loc = pos % P

    # bucket: lists[(c, r, t)] -> (src, loc, w) via sorting
    key = ((core * 2 + et) * NTILES + tl)
    order = np.argsort(key, kind="stable")
    key_s = key[order]
    src_s = src[order]
    loc_s = loc[order]
    w_s = w[order]
    nbuckets = N_CORES * 2 * NTILES
    counts = np.bincount(key_s, minlength=nbuckets)
    starts = np.concatenate([[0], np.cumsum(counts)])

    cmax = counts.reshape(N_CORES, 2, NTILES).max(axis=(0, 2))
    k0 = int(np.ceil(cmax[0] / P))
    k1 = int(np.ceil(cmax[1] / P))
    ks = [k0, k1, 1]
    nch = NTILES * (k0 + k1 + 1)

    offs = np.zeros((N_CORES, P, nch), np.int32)
    dstloc = np.zeros((N_CORES, P, nch), np.float32)
    wcol = np.zeros((N_CORES, P, nch), np.float32)

    for c in range(N_CORES):
        for r in range(2):
            kr = ks[r]
            base = NTILES * sum(ks[:r])
            for t in range(NTILES):
                b = (c * 2 + r) * NTILES + t
                s0, s1 = starts[b], starts[b + 1]
                n = s1 - s0
                # slot s (0..kr*128) -> (partition s%128, chunk s//128)
                buf_o = np.zeros(kr * P, np.int32)
                buf_d = np.zeros(kr * P, np.float32)
                buf_w = np.zeros(kr * P, np.float32)
                buf_o[:n] = src_s[s0:s1]
                buf_d[:n] = loc_s[s0:s1]
                buf_w[:n] = w_s[s0:s1]
                cc = base + t * kr
                offs[c, :, cc:cc + kr] = buf_o.reshape(kr, P).T
                dstloc[c, :, cc:cc + kr] = buf_d.reshape(kr, P).T
                wcol[c, :, cc:cc + kr] = buf_w.reshape(kr, P).T
        # relation 2: self-edges, 1 chunk per tile
        base2 = NTILES * (k0 + k1)
        for t in range(NTILES):
            lo = t * P
            hi = min(lo + P, NODES_PER_CORE)
            n = hi - lo
            buf_o = np.zeros(P, np.int32)
            buf_d = np.zeros(P, np.float32)
            buf_w = np.zeros(P, np.float32)
            buf_o[:n] = c * NODES_PER_CORE + lo + np.arange(n)
            buf_d[:n] = np.arange(n)
            buf_w[:n] = 1.0
            cc = base2 + t
            offs[c, :, cc] = buf_o
            dstloc[c, :, cc] = buf_d
            wcol[c, :, cc] = buf_w
    return k0, k1, offs, dstloc, wcol


def _run_layer(nc, table, offs, dstloc, wcol, w0, w1, wroot):
    iota = np.tile(np.arange(P, dtype=np.float32)[None, :], (P, 1))
    wmat = np.concatenate([w0, w1, wroot], axis=1).astype(np.float32)
    ins = [{
        "table": np.ascontiguousarray(table, np.float32),
        "offs": offs[c], "dstloc": dstloc[c], "wcol": wcol[c],
        "iota": iota, "wmat": wmat,
    } for c in range(N_CORES)]
    res = run_bass_kernel_spmd(nc, ins, core_ids=list(range(N_CORES)))
    aggr = np.empty((N, D), np.float32)
    for c in range(N_CORES):
        sl = res.results[c]["out"]  # [128 feat, 6272 pos] transposed slice
        aggr[c * NODES_PER_CORE:(c + 1) * NODES_PER_CORE] = \
            sl[:, :NODES_PER_CORE].T
    return aggr


def _layernorm(x, g, b):
    mu = x.mean(axis=-1, keepdims=True)
    var = np.square(x - mu).mean(axis=-1, keepdims=True)
    return (x - mu) / np.sqrt(var + EPS_LN) * g + b


def kernel(user_indices, item_indices, edge_index, edge_type, edge_weight,
           emb, W1_rel, W1_root, b1, g1, be1, W2_rel, W2_root, b2,
           mW1, mb1, mW2, mb2, mW3, mb3, oW, ob):
    user_indices = np.asarray(user_indices)
    item_indices = np.asarray(item_indices)
    edge_index = np.asarray(edge_index)
    edge_type = np.asarray(edge_type)
    edge_weight = np.asarray(edge_weight)
    emb = np.asarray(emb, np.float32)

    k0, k1, offs, dstloc, wcol = _prep_edges(edge_index, edge_type, edge_weight)
    key = (k0, k1)
    if key not in _compiled:
        _compiled[key] = _build_program(k0, k1)
    nc = _compiled[key]

    # Layer 1
    aggr1 = _run_layer(nc, emb, offs, dstloc, wcol,
                       np.asarray(W1_rel[0]), np.asarray(W1_rel[1]),
                       np.asarray(W1_root))
    h = np.maximum(aggr1 + np.asarray(b1)[None, :], 0.0)
    h = _layernorm(h, np.asarray(g1)[None, :], np.asarray(be1)[None, :])

    # Layer 2
    h2 = _run_layer(nc, h, offs, dstloc, wcol,
                    np.asarray(W2_rel[0]), np.asarray(W2_rel[1]),
                    np.asarray(W2_root))
    h2 = h2 + np.asarray(b2)[None, :]

    # Head (host, exact fp32 math mirroring the reference)
    u = h2[user_indices]
    it = h2[item_indices]
    un = u / np.maximum(np.linalg.norm(u, axis=-1, keepdims=True), EPS_NORM)
    itn = it / np.maximum(np.linalg.norm(it, axis=-1, keepdims=True), EPS_NORM)
    gmf = un * itn
    z = np.concatenate([u, it], axis=-1)
    z = np.maximum(z @ np.asarray(mW1) + np.asarray(mb1), 0.0)
    z = np.maximum(z @ np.asarray(mW2) + np.asarray(mb2), 0.0)
    z = np.maximum(z @ np.asarray(mW3) + np.asarray(mb3), 0.0)
    final = np.concatenate([gmf, z], axis=-1)
    score = (final @ np.asarray(oW) + np.asarray(ob)).squeeze(-1)
    return score.astype(np.float32)



# revision 2
# speedup vs baseline: 3.2915x; 3.2915x over previous
"""Trainium2 Bass kernel for nn_NeuralCF (2-layer RGCN + NeuralCF head), v2.

Strategy (8 NeuronCores, SPMD, dst-sharded):
  - Core c owns nodes [c*6250, (c+1)*6250), padded to 49 tiles of 128.
  - bf16 on device; PSUM fp32; output slice fp32.
  - Gather x[src] rows with dma_gather (SWDGE) on 4 rotating queues --
    one instruction per (tile, table-half), ~2.9ns/row descriptor rate.
    int16 gather indices force splitting the node table into two DRAM
    halves of 25088 rows.
  - Weighted one-hot per tile built by two broadcast tensor_tensor ops;
    one 128^3 bf16 matmul per 128-edge chunk accumulates A_r^T in PSUM.
  - Root term from a host-transposed slice of the table (no gather);
    stage 2 applies W_r / W_root per tile.
  - Host: edge bucketing/sorting (once), bias/relu/layernorm between the
    two device launches, small MLP head at the end.
"""
import numpy as np
import ml_dtypes

import concourse.bacc as bacc
import concourse.bass as bass
import concourse.mybir as mybir
import concourse.tile as tile
from concourse.bass_utils import run_bass_kernel_spmd

N = 50000
E = 1600000
D = 128
R = 2
B = 16384
EPS_LN = 1e-5
EPS_NORM = 1e-12

N_CORES = 8
NODES_PER_CORE = 6250
NTILES = 49
SLOTS = NTILES * 128   # 6272
P = 128
HALF = 25088           # rows per table half (int16-addressable)
N_PAD = 2 * HALF       # 50176 >= 43750+6272 (core 7 transposed slice)

BF16 = ml_dtypes.bfloat16

_compiled = {}


def _build_program(k, bases, nch):
    """k: [NTILES, 2 halves, R] chunk counts; bases: per-tile first column."""
    nc = bacc.Bacc("TRN2", target_bir_lowering=False, debug=False,
                   num_devices=N_CORES, num_swdge_queues=4)
    tlo = nc.dram_tensor("tlo", [HALF, D], mybir.dt.bfloat16, kind="ExternalInput")
    thi = nc.dram_tensor("thi", [HALF, D], mybir.dt.bfloat16, kind="ExternalInput")
    idxs = nc.dram_tensor("idxs", [P, nch * 8], mybir.dt.int16,
                          kind="ExternalInput")
    dstloc = nc.dram_tensor("dstloc", [P, nch], mybir.dt.bfloat16,
                            kind="ExternalInput")
    wcol = nc.dram_tensor("wcol", [P, nch], mybir.dt.bfloat16,
                          kind="ExternalInput")
    iota = nc.dram_tensor("iota", [P, P], mybir.dt.bfloat16, kind="ExternalInput")
    wmat = nc.dram_tensor("wmat", [P, 3 * P], mybir.dt.bfloat16,
                          kind="ExternalInput")
    xlocT = nc.dram_tensor("xlocT", [P, SLOTS], mybir.dt.bfloat16,
                           kind="ExternalInput")
    out = nc.dram_tensor("out", [P, SLOTS], mybir.dt.float32,
                         kind="ExternalOutput")

    kmax = int(max(k[t].sum() for t in range(NTILES)))
    qn = 0

    with tile.TileContext(nc) as tc:
        with (
            tc.tile_pool(name="const", bufs=1) as cpool,
            tc.tile_pool(name="xs", bufs=3) as xspool,
            tc.tile_pool(name="oh", bufs=3) as ohpool,
            tc.tile_pool(name="ar", bufs=4) as arpool,
            tc.tile_pool(name="ps", bufs=4, space="PSUM") as pspool,
            tc.tile_pool(name="ps2", bufs=2, space="PSUM") as ps2pool,
            tc.tile_pool(name="outT", bufs=1) as outpool,
        ):
            idx_s = cpool.tile([P, nch * 8], mybir.dt.int16)
            nc.sync.dma_start(idx_s[:], idxs[:, :])
            dst_s = cpool.tile([P, nch], mybir.dt.bfloat16)
            nc.sync.dma_start(dst_s[:], dstloc[:, :])
            w_s = cpool.tile([P, nch], mybir.dt.bfloat16)
            nc.sync.dma_start(w_s[:], wcol[:, :])
            iota_s = cpool.tile([P, P], mybir.dt.bfloat16)
            nc.sync.dma_start(iota_s[:], iota[:, :])
            wm_s = cpool.tile([P, 3 * P], mybir.dt.bfloat16)
            nc.sync.dma_start(wm_s[:], wmat[:, :])
            xT_s = cpool.tile([P, SLOTS], mybir.dt.bfloat16)
            nc.sync.dma_start(xT_s[:], xlocT[:, :])

            out_t = outpool.tile([P, SLOTS], mybir.dt.float32)

            for t in range(NTILES):
                klo = int(k[t, 0].sum())
                khi = int(k[t, 1].sum())
                kk = klo + khi
                c0 = bases[t]
                xs = xspool.tile([P, kmax * P], mybir.dt.bfloat16, tag="xs")
                for tab, cc0, kc in ((tlo, c0, klo), (thi, c0 + klo, khi)):
                    if kc == 0:
                        continue
                    off = (cc0 - c0) * P
                    nc.gpsimd.dma_gather(
                        xs[:, off:off + kc * P].rearrange(
                            "p (c q) -> p c q", q=P),
                        tab[:, :],
                        idx_s[:, cc0 * 8:(cc0 + kc) * 8],
                        kc * P, kc * P, P,
                        single_packet=False, queue_num=qn)
                    qn = (qn + 1) % 4
                oh = ohpool.tile([P, kmax * P], mybir.dt.bfloat16, tag="oh")
                oh3 = oh[:, :kk * P].rearrange("p (j q) -> p j q", j=kk)
                nc.vector.tensor_tensor(
                    out=oh3,
                    in0=iota_s[:].unsqueeze(1).broadcast_to([P, kk, P]),
                    in1=dst_s[:, c0:c0 + kk].unsqueeze(2).broadcast_to(
                        [P, kk, P]),
                    op=mybir.AluOpType.is_equal)
                nc.vector.tensor_tensor(
                    out=oh3, in0=oh3,
                    in1=w_s[:, c0:c0 + kk].unsqueeze(2).broadcast_to(
                        [P, kk, P]),
                    op=mybir.AluOpType.mult)

                # chunk ranges (tile-local) per relation: [lo-r0][lo-r1][hi-r0][hi-r1]
                r_ranges = [[], []]
                o = 0
                for h in range(2):
                    for r in range(R):
                        kn = int(k[t, h, r])
                        if kn:
                            r_ranges[r].append((o, o + kn))
                        o += kn
                psum2 = ps2pool.tile([P, P], mybir.dt.float32, space="PSUM")
                first2 = True
                for r in range(R):
                    spans = r_ranges[r]
                    if not spans:
                        continue
                    chunks = [j for a, b in spans for j in range(a, b)]
                    psum = pspool.tile([P, P], mybir.dt.float32, space="PSUM")
                    for i, j in enumerate(chunks):
                        nc.tensor.matmul(psum[:], lhsT=xs[:, j * P:(j + 1) * P],
                                         rhs=oh[:, j * P:(j + 1) * P],
                                         start=(i == 0),
                                         stop=(i == len(chunks) - 1))
                    ar = arpool.tile([P, P], mybir.dt.bfloat16, tag="ar")
                    nc.scalar.copy(out=ar[:], in_=psum[:])
                    nc.tensor.matmul(psum2[:], lhsT=wm_s[:, r * P:(r + 1) * P],
                                     rhs=ar[:], start=first2, stop=False)
                    first2 = False
                nc.tensor.matmul(psum2[:], lhsT=wm_s[:, 2 * P:3 * P],
                                 rhs=xT_s[:, t * P:(t + 1) * P],
                                 start=first2, stop=True)
                nc.scalar.copy(out=out_t[:, t * P:(t + 1) * P], in_=psum2[:])

            nc.sync.dma_start(out[:, :], out_t[:])

    nc.compile()
    return nc


def _prep_edges(edge_index, edge_type, edge_weight):
    """Bucket edges by (dst tile, src half, relation); sort by src in bucket.

    Returns (k[NTILES,2,R], bases, nch, idxs[8,128,nch*8] i16,
             dstloc[8,128,nch] bf16, wcol[8,128,nch] bf16).
    Column layout per tile: [lo-r0][lo-r1][hi-r0][hi-r1].
    Slot s in a bucket -> (partition s % 128, chunk s // 128); gather idx
    for global slot g = col*128 + p lives at idxs[16*rep + g%16, g//16].
    """
    src = edge_index[0].astype(np.int64)
    dst = edge_index[1].astype(np.int64)
    et = edge_type.astype(np.int64)
    w = edge_weight.astype(np.float32)

    core = dst // NODES_PER_CORE
    pos = dst % NODES_PER_CORE
    tl = pos // P
    loc = pos % P
    half = (src >= HALF).astype(np.int64)

    bucket = (((core * NTILES + tl) * 2 + half) * R + et)
    order = np.lexsort((src, bucket))
    bucket_s = bucket[order]
    src_s = src[order]
    loc_s = loc[order]
    w_s = w[order]

    nbuckets = N_CORES * NTILES * 2 * R
    counts = np.bincount(bucket_s, minlength=nbuckets)
    starts = np.concatenate([[0], np.cumsum(counts)])
    rank = np.arange(E, dtype=np.int64) - starts[bucket_s]

    cnt4 = counts.reshape(N_CORES, NTILES, 2, R)
    k = np.maximum(1, -(-cnt4 // P)).max(axis=0)     # [NTILES, 2, R]
    per_tile = k.sum(axis=(1, 2))
    bases = np.concatenate([[0], np.cumsum(per_tile)]).astype(np.int64)
    nch = int(bases[NTILES])

    # chunk column for each edge
    t_e = (bucket_s // (2 * R)) % NTILES
    h_e = (bucket_s // R) % 2
    r_e = bucket_s % R
    c_e = core[order]
    suboff = np.zeros((NTILES, 2, R), np.int64)
    suboff[:, 0, 1] = k[:, 0, 0]
    suboff[:, 1, 0] = k[:, 0, 0] + k[:, 0, 1]
    suboff[:, 1, 1] = k[:, 0, 0] + k[:, 0, 1] + k[:, 1, 0]
    col = bases[t_e] + suboff[t_e, h_e, r_e] + rank // P
    p_e = rank % P

    dstloc = np.zeros((N_CORES, P, nch), BF16)
    wcol = np.zeros((N_CORES, P, nch), BF16)
    dstloc[c_e, p_e, col] = loc_s.astype(BF16)
    wcol[c_e, p_e, col] = w_s.astype(BF16)

    idxs = np.zeros((N_CORES, P, nch * 8), np.int16)
    g = col * P + p_e
    src_reb = (src_s - h_e * HALF).astype(np.int16)
    for rep in range(8):
        idxs[c_e, 16 * rep + g % 16, g // 16] = src_reb
    return k, bases, nch, idxs, dstloc, wcol


def _run_layer(nc, table_pad, prep, wmat):
    _, _, _, idxs, dstloc, wcol = prep
    iota = np.tile(np.arange(P, dtype=np.float32)[None, :], (P, 1)).astype(BF16)
    tlo = table_pad[:HALF]
    thi = table_pad[HALF:]
    ins = []
    for c in range(N_CORES):
        c0 = c * NODES_PER_CORE
        xlocT = np.ascontiguousarray(table_pad[c0:c0 + SLOTS].T)
        ins.append({
            "tlo": tlo, "thi": thi, "idxs": idxs[c], "dstloc": dstloc[c],
            "wcol": wcol[c], "iota": iota, "wmat": wmat, "xlocT": xlocT,
        })
    res = run_bass_kernel_spmd(nc, ins, core_ids=list(range(N_CORES)))
    aggr = np.empty((N, D), np.float32)
    for c in range(N_CORES):
        sl = res.results[c]["out"]  # [128 feat, 6272 pos]
        aggr[c * NODES_PER_CORE:(c + 1) * NODES_PER_CORE] = \
            sl[:, :NODES_PER_CORE].T
    return aggr


def _layernorm(x, g, b):
    mu = x.mean(axis=-1, keepdims=True)
    var = np.square(x - mu).mean(axis=-1, keepdims=True)
    return (x - mu) / np.sqrt(var + EPS_LN) * g + b


def _pad_bf16(x):
    out = np.zeros((N_PAD, D), BF16)
    out[:N] = x.astype(BF16)
    return out


def kernel(user_indices, item_indices, edge_index, edge_type, edge_weight,
           emb, W1_rel, W1_root, b1, g1, be1, W2_rel, W2_root, b2,
           mW1, mb1, mW2, mb2, mW3, mb3, oW, ob):
    user_indices = np.asarray(user_indices)
    item_indices = np.asarray(item_indices)
    edge_index = np.asarray(edge_index)
    edge_type = np.asarray(edge_type)
    edge_weight = np.asarray(edge_weight)
    emb = np.asarray(emb, np.float32)

    prep = _prep_edges(edge_index, edge_type, edge_weight)
    k, bases, nch = prep[0], prep[1], prep[2]
    key = tuple(k.ravel())
    if key not in _compiled:
        _compiled[key] = _build_program(k, bases, nch)
    nc = _compiled[key]

    w1 = np.concatenate([np.asarray(W1_rel[0]), np.asarray(W1_rel[1]),
                         np.asarray(W1_root)], axis=1).astype(BF16)
    w2 = np.concatenate([np.asarray(W2_rel[0]), np.asarray(W2_rel[1]),
                         np.asarray(W2_root)], axis=1).astype(BF16)

    aggr1 = _run_layer(nc, _pad_bf16(emb), prep, w1)
    h = np.maximum(aggr1 + np.asarray(b1)[None, :], 0.0)
    h = _layernorm(h, np.asarray(g1)[None, :], np.asarray(be1)[None, :])

    h2 = _run_layer(nc, _pad_bf16(h), prep, w2)
    h2 = h2 + np.asarray(b2)[None, :]

    u = h2[user_indices]
    it = h2[item_indices]
    un = u / np.maximum(np.linalg.norm(u, axis=-1, keepdims=True), EPS_NORM)
    itn = it / np.maximum(np.linalg.norm(it, axis=-1, keepdims=True), EPS_NORM)
    gmf = un * itn
    z = np.concatenate([u, it], axis=-1)
    z = np.maximum(z @ np.asarray(mW1) + np.asarray(mb1), 0.0)
    z = np.maximum(z @ np.asarray(mW2) + np.asarray(mb2), 0.0)
    z = np.maximum(z @ np.asarray(mW3) + np.asarray(mb3), 0.0)
    final = np.concatenate([gmf, z], axis=-1)
    score = (final @ np.asarray(oW) + np.asarray(ob)).squeeze(-1)
    return score.astype(np.float32)


# revision 3
# speedup vs baseline: 4.0691x; 1.2363x over previous
"""Trainium2 Bass kernel for nn_NeuralCF (2-layer RGCN + NeuralCF head), v2.

Strategy (8 NeuronCores, SPMD, dst-sharded):
  - Core c owns nodes [c*6250, (c+1)*6250), padded to 49 tiles of 128.
  - bf16 on device; PSUM fp32; output slice fp32.
  - Gather x[src] rows with dma_gather (SWDGE) on 4 rotating queues --
    one instruction per (tile, table-half), ~2.9ns/row descriptor rate.
    int16 gather indices force splitting the node table into two DRAM
    halves of 25088 rows.
  - Weighted one-hot per tile built by two broadcast tensor_tensor ops;
    one 128^3 bf16 matmul per 128-edge chunk accumulates A_r^T in PSUM.
  - Root term from a host-transposed slice of the table (no gather);
    stage 2 applies W_r / W_root per tile.
  - Host: edge bucketing/sorting (once), bias/relu/layernorm between the
    two device launches, small MLP head at the end.
"""
import numpy as np
import ml_dtypes

import concourse.bacc as bacc
import concourse.bass as bass
import concourse.mybir as mybir
import concourse.tile as tile
from concourse.bass_utils import run_bass_kernel_spmd

N = 50000
E = 1600000
D = 128
R = 2
B = 16384
EPS_LN = 1e-5
EPS_NORM = 1e-12

N_CORES = 8
NODES_PER_CORE = 6250
NTILES = 49
SLOTS = NTILES * 128   # 6272
P = 128
HALF = 25088           # rows per table half (int16-addressable)
N_PAD = 2 * HALF       # 50176 >= 43750+6272 (core 7 transposed slice)

BF16 = ml_dtypes.bfloat16

_compiled = {}


def _build_program(k, bases, nch):
    """k: [NTILES, 2 halves, R] chunk counts; bases: per-tile first column."""
    nc = bacc.Bacc("TRN2", target_bir_lowering=False, debug=False,
                   num_devices=N_CORES, num_swdge_queues=4)
    tlo = nc.dram_tensor("tlo", [HALF, D], mybir.dt.bfloat16, kind="ExternalInput")
    thi = nc.dram_tensor("thi", [HALF, D], mybir.dt.bfloat16, kind="ExternalInput")
    idxs = nc.dram_tensor("idxs", [P, nch * 8], mybir.dt.int16,
                          kind="ExternalInput")
    dstloc = nc.dram_tensor("dstloc", [P, nch], mybir.dt.bfloat16,
                            kind="ExternalInput")
    wcol = nc.dram_tensor("wcol", [P, nch], mybir.dt.bfloat16,
                          kind="ExternalInput")
    iota = nc.dram_tensor("iota", [P, P], mybir.dt.bfloat16, kind="ExternalInput")
    wmat = nc.dram_tensor("wmat", [P, 3 * P], mybir.dt.bfloat16,
                          kind="ExternalInput")
    xlocT = nc.dram_tensor("xlocT", [P, SLOTS], mybir.dt.bfloat16,
                           kind="ExternalInput")
    out = nc.dram_tensor("out", [P, SLOTS], mybir.dt.float32,
                         kind="ExternalOutput")

    kmax = int(max(k[t].sum() for t in range(NTILES)))
    qn = 0

    with tile.TileContext(nc) as tc:
        with (
            tc.tile_pool(name="const", bufs=1) as cpool,
            tc.tile_pool(name="xs", bufs=3) as xspool,
            tc.tile_pool(name="oh", bufs=3) as ohpool,
            tc.tile_pool(name="ar", bufs=4) as arpool,
            tc.tile_pool(name="ps", bufs=4, space="PSUM") as pspool,
            tc.tile_pool(name="ps2", bufs=2, space="PSUM") as ps2pool,
            tc.tile_pool(name="outT", bufs=1) as outpool,
        ):
            idx_s = cpool.tile([P, nch * 8], mybir.dt.int16)
            nc.sync.dma_start(idx_s[:], idxs[:, :])
            dst_s = cpool.tile([P, nch], mybir.dt.bfloat16)
            nc.sync.dma_start(dst_s[:], dstloc[:, :])
            w_s = cpool.tile([P, nch], mybir.dt.bfloat16)
            nc.sync.dma_start(w_s[:], wcol[:, :])
            iota_s = cpool.tile([P, P], mybir.dt.bfloat16)
            nc.sync.dma_start(iota_s[:], iota[:, :])
            wm_s = cpool.tile([P, 3 * P], mybir.dt.bfloat16)
            nc.sync.dma_start(wm_s[:], wmat[:, :])
            xT_s = cpool.tile([P, SLOTS], mybir.dt.bfloat16)
            nc.sync.dma_start(xT_s[:], xlocT[:, :])

            out_t = outpool.tile([P, SLOTS], mybir.dt.float32)

            for t in range(NTILES):
                klo = int(k[t, 0].sum())
                khi = int(k[t, 1].sum())
                kk = klo + khi
                c0 = bases[t]
                xs = xspool.tile([P, kmax * P], mybir.dt.bfloat16, tag="xs")
                for tab, cc0, kc in ((tlo, c0, klo), (thi, c0 + klo, khi)):
                    if kc == 0:
                        continue
                    off = (cc0 - c0) * P
                    nc.gpsimd.dma_gather(
                        xs[:, off:off + kc * P].rearrange(
                            "p (c q) -> p c q", q=P),
                        tab[:, :],
                        idx_s[:, cc0 * 8:(cc0 + kc) * 8],
                        kc * P, kc * P, P,
                        single_packet=False, queue_num=qn)
                    qn = (qn + 1) % 4
                oh = ohpool.tile([P, kmax * P], mybir.dt.bfloat16, tag="oh")
                oh3 = oh[:, :kk * P].rearrange("p (j q) -> p j q", j=kk)
                nc.vector.tensor_tensor(
                    out=oh3,
                    in0=iota_s[:].unsqueeze(1).broadcast_to([P, kk, P]),
                    in1=dst_s[:, c0:c0 + kk].unsqueeze(2).broadcast_to(
                        [P, kk, P]),
                    op=mybir.AluOpType.is_equal)
                nc.vector.tensor_tensor(
                    out=oh3, in0=oh3,
                    in1=w_s[:, c0:c0 + kk].unsqueeze(2).broadcast_to(
                        [P, kk, P]),
                    op=mybir.AluOpType.mult)

                # chunk ranges (tile-local) per relation: [lo-r0][lo-r1][hi-r0][hi-r1]
                r_ranges = [[], []]
                o = 0
                for h in range(2):
                    for r in range(R):
                        kn = int(k[t, h, r])
                        if kn:
                            r_ranges[r].append((o, o + kn))
                        o += kn
                psum2 = ps2pool.tile([P, P], mybir.dt.float32, space="PSUM")
                first2 = True
                for r in range(R):
                    spans = r_ranges[r]
                    if not spans:
                        continue
                    chunks = [j for a, b in spans for j in range(a, b)]
                    psum = pspool.tile([P, P], mybir.dt.float32, space="PSUM")
                    for i, j in enumerate(chunks):
                        nc.tensor.matmul(psum[:], lhsT=xs[:, j * P:(j + 1) * P],
                                         rhs=oh[:, j * P:(j + 1) * P],
                                         start=(i == 0),
                                         stop=(i == len(chunks) - 1))
                    ar = arpool.tile([P, P], mybir.dt.bfloat16, tag="ar")
                    nc.scalar.copy(out=ar[:], in_=psum[:])
                    nc.tensor.matmul(psum2[:], lhsT=wm_s[:, r * P:(r + 1) * P],
                                     rhs=ar[:], start=first2, stop=False)
                    first2 = False
                nc.tensor.matmul(psum2[:], lhsT=wm_s[:, 2 * P:3 * P],
                                 rhs=xT_s[:, t * P:(t + 1) * P],
                                 start=first2, stop=True)
                nc.scalar.copy(out=out_t[:, t * P:(t + 1) * P], in_=psum2[:])

            nc.sync.dma_start(out[:, :], out_t[:])

    nc.compile()
    return nc


def _prep_edges(edge_index, edge_type, edge_weight, mask=None):
    """Bucket edges by (dst tile, src half, relation); sort by src in bucket.

    Returns (k[NTILES,2,R], bases, nch, idxs[8,128,nch*8] i16,
             dstloc[8,128,nch] bf16, wcol[8,128,nch] bf16).
    Column layout per tile: [lo-r0][lo-r1][hi-r0][hi-r1].
    Slot s in a bucket -> (partition s % 128, chunk s // 128); gather idx
    for global slot g = col*128 + p lives at idxs[16*rep + g%16, g//16].
    """
    src = edge_index[0].astype(np.int64)
    dst = edge_index[1].astype(np.int64)
    et = edge_type.astype(np.int64)
    w = edge_weight.astype(np.float32)
    if mask is not None:
        src, dst, et, w = src[mask], dst[mask], et[mask], w[mask]
    ne = len(src)

    core = dst // NODES_PER_CORE
    pos = dst % NODES_PER_CORE
    tl = pos // P
    loc = pos % P
    half = (src >= HALF).astype(np.int64)

    bucket = (((core * NTILES + tl) * 2 + half) * R + et)
    order = np.lexsort((src, bucket))
    bucket_s = bucket[order]
    src_s = src[order]
    loc_s = loc[order]
    w_s = w[order]

    nbuckets = N_CORES * NTILES * 2 * R
    counts = np.bincount(bucket_s, minlength=nbuckets)
    starts = np.concatenate([[0], np.cumsum(counts)])
    rank = np.arange(ne, dtype=np.int64) - starts[bucket_s]

    cnt4 = counts.reshape(N_CORES, NTILES, 2, R)
    k = np.maximum(1, -(-cnt4 // P)).max(axis=0)     # [NTILES, 2, R]
    per_tile = k.sum(axis=(1, 2))
    bases = np.concatenate([[0], np.cumsum(per_tile)]).astype(np.int64)
    nch = int(bases[NTILES])

    # chunk column for each edge
    t_e = (bucket_s // (2 * R)) % NTILES
    h_e = (bucket_s // R) % 2
    r_e = bucket_s % R
    c_e = core[order]
    suboff = np.zeros((NTILES, 2, R), np.int64)
    suboff[:, 0, 1] = k[:, 0, 0]
    suboff[:, 1, 0] = k[:, 0, 0] + k[:, 0, 1]
    suboff[:, 1, 1] = k[:, 0, 0] + k[:, 0, 1] + k[:, 1, 0]
    col = bases[t_e] + suboff[t_e, h_e, r_e] + rank // P
    p_e = rank % P

    dstloc = np.zeros((N_CORES, P, nch), BF16)
    wcol = np.zeros((N_CORES, P, nch), BF16)
    dstloc[c_e, p_e, col] = loc_s.astype(BF16)
    wcol[c_e, p_e, col] = w_s.astype(BF16)

    idxs = np.zeros((N_CORES, P, nch * 8), np.int16)
    g = col * P + p_e
    src_reb = (src_s - h_e * HALF).astype(np.int16)
    for rep in range(8):
        idxs[c_e, 16 * rep + g % 16, g // 16] = src_reb
    return k, bases, nch, idxs, dstloc, wcol


def _run_layer(nc, table_pad, prep, wmat):
    _, _, _, idxs, dstloc, wcol = prep
    iota = np.tile(np.arange(P, dtype=np.float32)[None, :], (P, 1)).astype(BF16)
    tlo = table_pad[:HALF]
    thi = table_pad[HALF:]
    ins = []
    for c in range(N_CORES):
        c0 = c * NODES_PER_CORE
        xlocT = np.ascontiguousarray(table_pad[c0:c0 + SLOTS].T)
        ins.append({
            "tlo": tlo, "thi": thi, "idxs": idxs[c], "dstloc": dstloc[c],
            "wcol": wcol[c], "iota": iota, "wmat": wmat, "xlocT": xlocT,
        })
    res = run_bass_kernel_spmd(nc, ins, core_ids=list(range(N_CORES)))
    aggr = np.empty((N, D), np.float32)
    for c in range(N_CORES):
        sl = res.results[c]["out"]  # [128 feat, 6272 pos]
        aggr[c * NODES_PER_CORE:(c + 1) * NODES_PER_CORE] = \
            sl[:, :NODES_PER_CORE].T
    return aggr


def _layernorm(x, g, b):
    mu = x.mean(axis=-1, keepdims=True)
    var = np.square(x - mu).mean(axis=-1, keepdims=True)
    return (x - mu) / np.sqrt(var + EPS_LN) * g + b


def _pad_bf16(x):
    out = np.zeros((N_PAD, D), BF16)
    out[:N] = x.astype(BF16)
    return out


def kernel(user_indices, item_indices, edge_index, edge_type, edge_weight,
           emb, W1_rel, W1_root, b1, g1, be1, W2_rel, W2_root, b2,
           mW1, mb1, mW2, mb2, mW3, mb3, oW, ob):
    user_indices = np.asarray(user_indices)
    item_indices = np.asarray(item_indices)
    edge_index = np.asarray(edge_index)
    edge_type = np.asarray(edge_type)
    edge_weight = np.asarray(edge_weight)
    emb = np.asarray(emb, np.float32)

    prep1 = _prep_edges(edge_index, edge_type, edge_weight)
    needed2 = np.zeros(N, bool)
    needed2[user_indices] = True
    needed2[item_indices] = True
    prep2 = _prep_edges(edge_index, edge_type, edge_weight,
                        mask=needed2[np.asarray(edge_index[1])])
    ncs = []
    for prep in (prep1, prep2):
        k, bases, nch = prep[0], prep[1], prep[2]
        key = tuple(k.ravel())
        if key not in _compiled:
            _compiled[key] = _build_program(k, bases, nch)
        ncs.append(_compiled[key])

    w1 = np.concatenate([np.asarray(W1_rel[0]), np.asarray(W1_rel[1]),
                         np.asarray(W1_root)], axis=1).astype(BF16)
    w2 = np.concatenate([np.asarray(W2_rel[0]), np.asarray(W2_rel[1]),
                         np.asarray(W2_root)], axis=1).astype(BF16)

    aggr1 = _run_layer(ncs[0], _pad_bf16(emb), prep1, w1)
    h = np.maximum(aggr1 + np.asarray(b1)[None, :], 0.0)
    h = _layernorm(h, np.asarray(g1)[None, :], np.asarray(be1)[None, :])

    h2 = _run_layer(ncs[1], _pad_bf16(h), prep2, w2)
    h2 = h2 + np.asarray(b2)[None, :]

    u = h2[user_indices]
    it = h2[item_indices]
    un = u / np.maximum(np.linalg.norm(u, axis=-1, keepdims=True), EPS_NORM)
    itn = it / np.maximum(np.linalg.norm(it, axis=-1, keepdims=True), EPS_NORM)
    gmf = un * itn
    z = np.concatenate([u, it], axis=-1)
    z = np.maximum(z @ np.asarray(mW1) + np.asarray(mb1), 0.0)
    z = np.maximum(z @ np.asarray(mW2) + np.asarray(mb2), 0.0)
    z = np.maximum(z @ np.asarray(mW3) + np.asarray(mb3), 0.0)
    final = np.concatenate([gmf, z], axis=-1)
    score = (final @ np.asarray(oW) + np.asarray(ob)).squeeze(-1)
    return score.astype(np.float32)


# revision 4
# speedup vs baseline: 4.5512x; 1.1185x over previous
"""Trainium2 Bass kernel for nn_NeuralCF (2-layer RGCN + NeuralCF head), v2.

Strategy (8 NeuronCores, SPMD, dst-sharded):
  - Core c owns nodes [c*6250, (c+1)*6250), padded to 49 tiles of 128.
  - bf16 on device; PSUM fp32; output slice fp32.
  - Gather x[src] rows with dma_gather (SWDGE) on 4 rotating queues --
    one instruction per (tile, table-half), ~2.9ns/row descriptor rate.
    int16 gather indices force splitting the node table into two DRAM
    halves of 25088 rows.
  - Weighted one-hot per tile built by two broadcast tensor_tensor ops;
    one 128^3 bf16 matmul per 128-edge chunk accumulates A_r^T in PSUM.
  - Root term from a host-transposed slice of the table (no gather);
    stage 2 applies W_r / W_root per tile.
  - Host: edge bucketing/sorting (once), bias/relu/layernorm between the
    two device launches, small MLP head at the end.
"""
import numpy as np
import ml_dtypes

import concourse.bacc as bacc
import concourse.bass as bass
import concourse.mybir as mybir
import concourse.tile as tile
from concourse.bass_utils import run_bass_kernel_spmd

N = 50000
E = 1600000
D = 128
R = 2
B = 16384
EPS_LN = 1e-5
EPS_NORM = 1e-12

N_CORES = 8
NODES_PER_CORE = 6250
NTILES = 49
SLOTS = NTILES * 128   # 6272
P = 128
HALF = 25088           # rows per table half (int16-addressable)
N_PAD = 2 * HALF       # 50176 >= 43750+6272 (core 7 transposed slice)

BF16 = ml_dtypes.bfloat16

_compiled = {}


def _build_program(k, bases, nch):
    """k: [NTILES, 2 halves, R] chunk counts; bases: per-tile first column."""
    nc = bacc.Bacc("TRN2", target_bir_lowering=False, debug=False,
                   num_devices=N_CORES, num_swdge_queues=4)
    tlo = nc.dram_tensor("tlo", [HALF, D], mybir.dt.bfloat16, kind="ExternalInput")
    thi = nc.dram_tensor("thi", [HALF, D], mybir.dt.bfloat16, kind="ExternalInput")
    idxs = nc.dram_tensor("idxs", [P, nch * 8], mybir.dt.int16,
                          kind="ExternalInput")
    dstloc = nc.dram_tensor("dstloc", [P, nch], mybir.dt.bfloat16,
                            kind="ExternalInput")
    wcol = nc.dram_tensor("wcol", [P, nch], mybir.dt.bfloat16,
                          kind="ExternalInput")
    iota = nc.dram_tensor("iota", [P, P], mybir.dt.bfloat16, kind="ExternalInput")
    wmat = nc.dram_tensor("wmat", [P, 3 * P], mybir.dt.bfloat16,
                          kind="ExternalInput")
    xlocT = nc.dram_tensor("xlocT", [P, SLOTS], mybir.dt.bfloat16,
                           kind="ExternalInput")
    out = nc.dram_tensor("out", [P, SLOTS], mybir.dt.float32,
                         kind="ExternalOutput")

    kmax = int(max(k[t].sum() for t in range(NTILES)))
    qn = 0

    with tile.TileContext(nc) as tc:
        with (
            tc.tile_pool(name="const", bufs=1) as cpool,
            tc.tile_pool(name="xs", bufs=4) as xspool,
            tc.tile_pool(name="oh", bufs=4) as ohpool,
            tc.tile_pool(name="ar", bufs=4) as arpool,
            tc.tile_pool(name="ps", bufs=4, space="PSUM") as pspool,
            tc.tile_pool(name="ps2", bufs=2, space="PSUM") as ps2pool,
            tc.tile_pool(name="outT", bufs=1) as outpool,
        ):
            idx_s = cpool.tile([P, nch * 8], mybir.dt.int16)
            nc.sync.dma_start(idx_s[:], idxs[:, :])
            dst_s = cpool.tile([P, nch], mybir.dt.bfloat16)
            nc.sync.dma_start(dst_s[:], dstloc[:, :])
            w_s = cpool.tile([P, nch], mybir.dt.bfloat16)
            nc.sync.dma_start(w_s[:], wcol[:, :])
            iota_s = cpool.tile([P, P], mybir.dt.bfloat16)
            nc.sync.dma_start(iota_s[:], iota[:, :])
            wm_s = cpool.tile([P, 3 * P], mybir.dt.bfloat16)
            nc.sync.dma_start(wm_s[:], wmat[:, :])
            xT_s = cpool.tile([P, SLOTS], mybir.dt.bfloat16)
            nc.sync.dma_start(xT_s[:], xlocT[:, :])

            out_t = outpool.tile([P, SLOTS], mybir.dt.float32)

            for t in range(NTILES):
                klo = int(k[t, 0].sum())
                khi = int(k[t, 1].sum())
                kk = klo + khi
                c0 = bases[t]
                xs = xspool.tile([P, kmax * P], mybir.dt.bfloat16, tag="xs")
                for tab, cc0, kc in ((tlo, c0, klo), (thi, c0 + klo, khi)):
                    if kc == 0:
                        continue
                    off = (cc0 - c0) * P
                    nc.gpsimd.dma_gather(
                        xs[:, off:off + kc * P].rearrange(
                            "p (c q) -> p c q", q=P),
                        tab[:, :],
                        idx_s[:, cc0 * 8:(cc0 + kc) * 8],
                        kc * P, kc * P, P,
                        single_packet=False, queue_num=qn)
                    qn = (qn + 1) % 4
                oh = ohpool.tile([P, kmax * P], mybir.dt.bfloat16, tag="oh")
                oh3 = oh[:, :kk * P].rearrange("p (j q) -> p j q", j=kk)
                nc.vector.tensor_tensor(
                    out=oh3,
                    in0=iota_s[:].unsqueeze(1).broadcast_to([P, kk, P]),
                    in1=dst_s[:, c0:c0 + kk].unsqueeze(2).broadcast_to(
                        [P, kk, P]),
                    op=mybir.AluOpType.is_equal)
                nc.vector.tensor_tensor(
                    out=oh3, in0=oh3,
                    in1=w_s[:, c0:c0 + kk].unsqueeze(2).broadcast_to(
                        [P, kk, P]),
                    op=mybir.AluOpType.mult)

                # chunk ranges (tile-local) per relation: [lo-r0][lo-r1][hi-r0][hi-r1]
                r_ranges = [[], []]
                o = 0
                for h in range(2):
                    for r in range(R):
                        kn = int(k[t, h, r])
                        if kn:
                            r_ranges[r].append((o, o + kn))
                        o += kn
                psum2 = ps2pool.tile([P, P], mybir.dt.float32, space="PSUM")
                first2 = True
                for r in range(R):
                    spans = r_ranges[r]
                    if not spans:
                        continue
                    chunks = [j for a, b in spans for j in range(a, b)]
                    psum = pspool.tile([P, P], mybir.dt.float32, space="PSUM")
                    for i, j in enumerate(chunks):
                        nc.tensor.matmul(psum[:], lhsT=xs[:, j * P:(j + 1) * P],
                                         rhs=oh[:, j * P:(j + 1) * P],
                                         start=(i == 0),
                                         stop=(i == len(chunks) - 1))
                    ar = arpool.tile([P, P], mybir.dt.bfloat16, tag="ar")
                    nc.scalar.copy(out=ar[:], in_=psum[:])
                    nc.tensor.matmul(psum2[:], lhsT=wm_s[:, r * P:(r + 1) * P],
                                     rhs=ar[:], start=first2, stop=False)
                    first2 = False
                nc.tensor.matmul(psum2[:], lhsT=wm_s[:, 2 * P:3 * P],
                                 rhs=xT_s[:, t * P:(t + 1) * P],
                                 start=first2, stop=True)
                nc.scalar.copy(out=out_t[:, t * P:(t + 1) * P], in_=psum2[:])

            nc.sync.dma_start(out[:, :], out_t[:])

    nc.compile()
    return nc


def _prep_edges(edge_index, edge_type, edge_weight, mask=None):
    """Bucket edges by (dst tile, src half, relation); sort by src in bucket.

    Returns (k[NTILES,2,R], bases, nch, idxs[8,128,nch*8] i16,
             dstloc[8,128,nch] bf16, wcol[8,128,nch] bf16).
    Column layout per tile: [lo-r0][lo-r1][hi-r0][hi-r1].
    Slot s in a bucket -> (partition s % 128, chunk s // 128); gather idx
    for global slot g = col*128 + p lives at idxs[16*rep + g%16, g//16].
    """
    src = edge_index[0].astype(np.int64)
    dst = edge_index[1].astype(np.int64)
    et = edge_type.astype(np.int64)
    w = edge_weight.astype(np.float32)
    if mask is not None:
        src, dst, et, w = src[mask], dst[mask], et[mask], w[mask]
    ne = len(src)

    core = dst // NODES_PER_CORE
    pos = dst % NODES_PER_CORE
    tl = pos // P
    loc = pos % P
    half = (src >= HALF).astype(np.int64)

    bucket = (((core * NTILES + tl) * 2 + half) * R + et)
    order = np.lexsort((src, bucket))
    bucket_s = bucket[order]
    src_s = src[order]
    loc_s = loc[order]
    w_s = w[order]

    nbuckets = N_CORES * NTILES * 2 * R
    counts = np.bincount(bucket_s, minlength=nbuckets)
    starts = np.concatenate([[0], np.cumsum(counts)])
    rank = np.arange(ne, dtype=np.int64) - starts[bucket_s]

    cnt4 = counts.reshape(N_CORES, NTILES, 2, R)
    k = np.maximum(1, -(-cnt4 // P)).max(axis=0)     # [NTILES, 2, R]
    per_tile = k.sum(axis=(1, 2))
    bases = np.concatenate([[0], np.cumsum(per_tile)]).astype(np.int64)
    nch = int(bases[NTILES])

    # chunk column for each edge
    t_e = (bucket_s // (2 * R)) % NTILES
    h_e = (bucket_s // R) % 2
    r_e = bucket_s % R
    c_e = core[order]
    suboff = np.zeros((NTILES, 2, R), np.int64)
    suboff[:, 0, 1] = k[:, 0, 0]
    suboff[:, 1, 0] = k[:, 0, 0] + k[:, 0, 1]
    suboff[:, 1, 1] = k[:, 0, 0] + k[:, 0, 1] + k[:, 1, 0]
    col = bases[t_e] + suboff[t_e, h_e, r_e] + rank // P
    p_e = rank % P

    dstloc = np.zeros((N_CORES, P, nch), BF16)
    wcol = np.zeros((N_CORES, P, nch), BF16)
    dstloc[c_e, p_e, col] = loc_s.astype(BF16)
    wcol[c_e, p_e, col] = w_s.astype(BF16)

    idxs = np.zeros((N_CORES, P, nch * 8), np.int16)
    g = col * P + p_e
    src_reb = (src_s - h_e * HALF).astype(np.int16)
    for rep in range(8):
        idxs[c_e, 16 * rep + g % 16, g // 16] = src_reb
    return k, bases, nch, idxs, dstloc, wcol


def _run_layer(nc, table_pad, prep, wmat):
    _, _, _, idxs, dstloc, wcol = prep
    iota = np.tile(np.arange(P, dtype=np.float32)[None, :], (P, 1)).astype(BF16)
    tlo = table_pad[:HALF]
    thi = table_pad[HALF:]
    ins = []
    for c in range(N_CORES):
        c0 = c * NODES_PER_CORE
        xlocT = np.ascontiguousarray(table_pad[c0:c0 + SLOTS].T)
        ins.append({
            "tlo": tlo, "thi": thi, "idxs": idxs[c], "dstloc": dstloc[c],
            "wcol": wcol[c], "iota": iota, "wmat": wmat, "xlocT": xlocT,
        })
    res = run_bass_kernel_spmd(nc, ins, core_ids=list(range(N_CORES)))
    aggr = np.empty((N, D), np.float32)
    for c in range(N_CORES):
        sl = res.results[c]["out"]  # [128 feat, 6272 pos]
        aggr[c * NODES_PER_CORE:(c + 1) * NODES_PER_CORE] = \
            sl[:, :NODES_PER_CORE].T
    return aggr


def _layernorm(x, g, b):
    mu = x.mean(axis=-1, keepdims=True)
    var = np.square(x - mu).mean(axis=-1, keepdims=True)
    return (x - mu) / np.sqrt(var + EPS_LN) * g + b


def _pad_bf16(x):
    out = np.zeros((N_PAD, D), BF16)
    out[:N] = x.astype(BF16)
    return out


def kernel(user_indices, item_indices, edge_index, edge_type, edge_weight,
           emb, W1_rel, W1_root, b1, g1, be1, W2_rel, W2_root, b2,
           mW1, mb1, mW2, mb2, mW3, mb3, oW, ob):
    user_indices = np.asarray(user_indices)
    item_indices = np.asarray(item_indices)
    edge_index = np.asarray(edge_index)
    edge_type = np.asarray(edge_type)
    edge_weight = np.asarray(edge_weight)
    emb = np.asarray(emb, np.float32)

    prep1 = _prep_edges(edge_index, edge_type, edge_weight)
    needed2 = np.zeros(N, bool)
    needed2[user_indices] = True
    needed2[item_indices] = True
    prep2 = _prep_edges(edge_index, edge_type, edge_weight,
                        mask=needed2[np.asarray(edge_index[1])])
    ncs = []
    for prep in (prep1, prep2):
        k, bases, nch = prep[0], prep[1], prep[2]
        key = tuple(k.ravel())
        if key not in _compiled:
            _compiled[key] = _build_program(k, bases, nch)
        ncs.append(_compiled[key])

    w1 = np.concatenate([np.asarray(W1_rel[0]), np.asarray(W1_rel[1]),
                         np.asarray(W1_root)], axis=1).astype(BF16)
    w2 = np.concatenate([np.asarray(W2_rel[0]), np.asarray(W2_rel[1]),
                         np.asarray(W2_root)], axis=1).astype(BF16)

    aggr1 = _run_layer(ncs[0], _pad_bf16(emb), prep1, w1)
    h = np.maximum(aggr1 + np.asarray(b1)[None, :], 0.0)
    h = _layernorm(h, np.asarray(g1)[None, :], np.asarray(be1)[None, :])

    h2 = _run_layer(ncs[1], _pad_bf16(h), prep2, w2)
    h2 = h2 + np.asarray(b2)[None, :]

    u = h2[user_indices]
    it = h2[item_indices]
    un = u / np.maximum(np.linalg.norm(u, axis=-1, keepdims=True), EPS_NORM)
    itn = it / np.maximum(np.linalg.norm(it, axis=-1, keepdims=True), EPS_NORM)
    gmf = un * itn
    z = np.concatenate([u, it], axis=-1)
    z = np.maximum(z @ np.asarray(mW1) + np.asarray(mb1), 0.0)
    z = np.maximum(z @ np.asarray(mW2) + np.asarray(mb2), 0.0)
    z = np.maximum(z @ np.asarray(mW3) + np.asarray(mb3), 0.0)
    final = np.concatenate([gmf, z], axis=-1)
    score = (final @ np.asarray(oW) + np.asarray(ob)).squeeze(-1)
    return score.astype(np.float32)


# revision 5
# speedup vs baseline: 5.2441x; 1.1522x over previous
"""Trainium2 Bass kernel for nn_NeuralCF (2-layer RGCN + NeuralCF head), v2.

Strategy (8 NeuronCores, SPMD, dst-sharded):
  - Core c owns nodes [c*6250, (c+1)*6250), padded to 49 tiles of 128.
  - bf16 on device; PSUM fp32; output slice fp32.
  - Gather x[src] rows with dma_gather (SWDGE) on 4 rotating queues --
    one instruction per (tile, table-half), ~2.9ns/row descriptor rate.
    int16 gather indices force splitting the node table into two DRAM
    halves of 25088 rows.
  - Weighted one-hot per tile built by two broadcast tensor_tensor ops;
    one 128^3 bf16 matmul per 128-edge chunk accumulates A_r^T in PSUM.
  - Root term from a host-transposed slice of the table (no gather);
    stage 2 applies W_r / W_root per tile.
  - Host: edge bucketing/sorting (once), bias/relu/layernorm between the
    two device launches, small MLP head at the end.
"""
import numpy as np
import ml_dtypes

import concourse.bacc as bacc
import concourse.bass as bass
import concourse.mybir as mybir
import concourse.tile as tile
from concourse.bass_utils import run_bass_kernel_spmd

N = 50000
E = 1600000
D = 128
R = 2
B = 16384
EPS_LN = 1e-5
EPS_NORM = 1e-12

N_CORES = 8
NODES_PER_CORE = 6250
NTILES = 49
SLOTS = NTILES * 128   # 6272
P = 128
HALF = 25088           # rows per table half (int16-addressable)
N_PAD = 2 * HALF       # 50176 >= 43750+6272 (core 7 transposed slice)

BF16 = ml_dtypes.bfloat16

_compiled = {}


def _build_program(k, bases, nch):
    """k: [NTILES, 2 halves, R] chunk counts; bases: per-tile first column."""
    nc = bacc.Bacc("TRN2", target_bir_lowering=False, debug=False,
                   num_devices=N_CORES, num_swdge_queues=4)
    tlo = nc.dram_tensor("tlo", [HALF, D], mybir.dt.bfloat16, kind="ExternalInput")
    thi = nc.dram_tensor("thi", [HALF, D], mybir.dt.bfloat16, kind="ExternalInput")
    idxs = nc.dram_tensor("idxs", [P, nch * 8], mybir.dt.int16,
                          kind="ExternalInput")
    dstloc = nc.dram_tensor("dstloc", [P, nch], mybir.dt.bfloat16,
                            kind="ExternalInput")
    wcol = nc.dram_tensor("wcol", [P, nch], mybir.dt.bfloat16,
                          kind="ExternalInput")
    iota = nc.dram_tensor("iota", [P, P], mybir.dt.bfloat16, kind="ExternalInput")
    wmat = nc.dram_tensor("wmat", [P, 3 * P], mybir.dt.bfloat16,
                          kind="ExternalInput")
    xlocT = nc.dram_tensor("xlocT", [P, SLOTS], mybir.dt.bfloat16,
                           kind="ExternalInput")
    out = nc.dram_tensor("out", [P, SLOTS], mybir.dt.float32,
                         kind="ExternalOutput")

    kmax = int(max(k[t].sum() for t in range(NTILES)))
    qn = 0

    with tile.TileContext(nc) as tc:
        with (
            tc.tile_pool(name="const", bufs=1) as cpool,
            tc.tile_pool(name="xs", bufs=4) as xspool,
            tc.tile_pool(name="oh", bufs=4) as ohpool,
            tc.tile_pool(name="ar", bufs=4) as arpool,
            tc.tile_pool(name="ps", bufs=4, space="PSUM") as pspool,
            tc.tile_pool(name="ps2", bufs=2, space="PSUM") as ps2pool,
            tc.tile_pool(name="outT", bufs=1) as outpool,
        ):
            h8 = (nch * 8) // 2
            hn = nch // 2
            idx_s = cpool.tile([P, nch * 8], mybir.dt.int16)
            nc.sync.dma_start(idx_s[:, :h8], idxs[:, :h8])
            nc.sync.dma_start(idx_s[:, h8:], idxs[:, h8:])
            dst_s = cpool.tile([P, nch], mybir.dt.bfloat16)
            nc.sync.dma_start(dst_s[:, :hn], dstloc[:, :hn])
            nc.sync.dma_start(dst_s[:, hn:], dstloc[:, hn:])
            w_s = cpool.tile([P, nch], mybir.dt.bfloat16)
            nc.sync.dma_start(w_s[:, :hn], wcol[:, :hn])
            nc.sync.dma_start(w_s[:, hn:], wcol[:, hn:])
            iota_s = cpool.tile([P, P], mybir.dt.bfloat16)
            nc.sync.dma_start(iota_s[:], iota[:, :])
            wm_s = cpool.tile([P, 3 * P], mybir.dt.bfloat16)
            nc.sync.dma_start(wm_s[:], wmat[:, :])
            xT_s = cpool.tile([P, SLOTS], mybir.dt.bfloat16)
            nc.sync.dma_start(xT_s[:], xlocT[:, :])

            out_t = outpool.tile([P, SLOTS], mybir.dt.float32)

            for t in range(NTILES):
                klo = int(k[t, 0].sum())
                khi = int(k[t, 1].sum())
                kk = klo + khi
                c0 = bases[t]
                xs = xspool.tile([P, kmax * P], mybir.dt.bfloat16, tag="xs")
                for tab, cc0, kc in ((tlo, c0, klo), (thi, c0 + klo, khi)):
                    if kc == 0:
                        continue
                    off = (cc0 - c0) * P
                    nc.gpsimd.dma_gather(
                        xs[:, off:off + kc * P].rearrange(
                            "p (c q) -> p c q", q=P),
                        tab[:, :],
                        idx_s[:, cc0 * 8:(cc0 + kc) * 8],
                        kc * P, kc * P, P,
                        single_packet=False, queue_num=qn)
                    qn = (qn + 1) % 4
                oh = ohpool.tile([P, kmax * P], mybir.dt.bfloat16, tag="oh")
                oh3 = oh[:, :kk * P].rearrange("p (j q) -> p j q", j=kk)
                nc.vector.tensor_tensor(
                    out=oh3,
                    in0=iota_s[:].unsqueeze(1).broadcast_to([P, kk, P]),
                    in1=dst_s[:, c0:c0 + kk].unsqueeze(2).broadcast_to(
                        [P, kk, P]),
                    op=mybir.AluOpType.is_equal)
                nc.vector.tensor_tensor(
                    out=oh3, in0=oh3,
                    in1=w_s[:, c0:c0 + kk].unsqueeze(2).broadcast_to(
                        [P, kk, P]),
                    op=mybir.AluOpType.mult)

                # chunk ranges (tile-local) per relation: [lo-r0][lo-r1][hi-r0][hi-r1]
                r_ranges = [[], []]
                o = 0
                for h in range(2):
                    for r in range(R):
                        kn = int(k[t, h, r])
                        if kn:
                            r_ranges[r].append((o, o + kn))
                        o += kn
                psum2 = ps2pool.tile([P, P], mybir.dt.float32, space="PSUM")
                first2 = True
                for r in range(R):
                    spans = r_ranges[r]
                    if not spans:
                        continue
                    chunks = [j for a, b in spans for j in range(a, b)]
                    psum = pspool.tile([P, P], mybir.dt.float32, space="PSUM")
                    for i, j in enumerate(chunks):
                        nc.tensor.matmul(psum[:], lhsT=xs[:, j * P:(j + 1) * P],
                                         rhs=oh[:, j * P:(j + 1) * P],
                                         start=(i == 0),
                                         stop=(i == len(chunks) - 1))
                    ar = arpool.tile([P, P], mybir.dt.bfloat16, tag="ar")
                    nc.scalar.copy(out=ar[:], in_=psum[:])
                    nc.tensor.matmul(psum2[:], lhsT=wm_s[:, r * P:(r + 1) * P],
                                     rhs=ar[:], start=first2, stop=False)
                    first2 = False
                nc.tensor.matmul(psum2[:], lhsT=wm_s[:, 2 * P:3 * P],
                                 rhs=xT_s[:, t * P:(t + 1) * P],
                                 start=first2, stop=True)
                nc.scalar.copy(out=out_t[:, t * P:(t + 1) * P], in_=psum2[:])
                nc.sync.dma_start(out[:, t * P:(t + 1) * P],
                                  out_t[:, t * P:(t + 1) * P])

    nc.compile()
    return nc


def _prep_edges(edge_index, edge_type, edge_weight, mask=None):
    """Bucket edges by (dst tile, src half, relation); sort by src in bucket.

    Returns (k[NTILES,2,R], bases, nch, idxs[8,128,nch*8] i16,
             dstloc[8,128,nch] bf16, wcol[8,128,nch] bf16).
    Column layout per tile: [lo-r0][lo-r1][hi-r0][hi-r1].
    Slot s in a bucket -> (partition s % 128, chunk s // 128); gather idx
    for global slot g = col*128 + p lives at idxs[16*rep + g%16, g//16].
    """
    src = edge_index[0].astype(np.int64)
    dst = edge_index[1].astype(np.int64)
    et = edge_type.astype(np.int64)
    w = edge_weight.astype(np.float32)
    if mask is not None:
        src, dst, et, w = src[mask], dst[mask], et[mask], w[mask]
    ne = len(src)

    core = dst // NODES_PER_CORE
    pos = dst % NODES_PER_CORE
    tl = pos // P
    loc = pos % P
    half = (src >= HALF).astype(np.int64)

    bucket = (((core * NTILES + tl) * 2 + half) * R + et)
    order = np.lexsort((src, bucket))
    bucket_s = bucket[order]
    src_s = src[order]
    loc_s = loc[order]
    w_s = w[order]

    nbuckets = N_CORES * NTILES * 2 * R
    counts = np.bincount(bucket_s, minlength=nbuckets)
    starts = np.concatenate([[0], np.cumsum(counts)])
    rank = np.arange(ne, dtype=np.int64) - starts[bucket_s]

    cnt4 = counts.reshape(N_CORES, NTILES, 2, R)
    k = np.maximum(1, -(-cnt4 // P)).max(axis=0)     # [NTILES, 2, R]
    per_tile = k.sum(axis=(1, 2))
    bases = np.concatenate([[0], np.cumsum(per_tile)]).astype(np.int64)
    nch = int(bases[NTILES])

    # chunk column for each edge
    t_e = (bucket_s // (2 * R)) % NTILES
    h_e = (bucket_s // R) % 2
    r_e = bucket_s % R
    c_e = core[order]
    suboff = np.zeros((NTILES, 2, R), np.int64)
    suboff[:, 0, 1] = k[:, 0, 0]
    suboff[:, 1, 0] = k[:, 0, 0] + k[:, 0, 1]
    suboff[:, 1, 1] = k[:, 0, 0] + k[:, 0, 1] + k[:, 1, 0]
    col = bases[t_e] + suboff[t_e, h_e, r_e] + rank // P
    p_e = rank % P

    dstloc = np.zeros((N_CORES, P, nch), BF16)
    wcol = np.zeros((N_CORES, P, nch), BF16)
    dstloc[c_e, p_e, col] = loc_s.astype(BF16)
    wcol[c_e, p_e, col] = w_s.astype(BF16)

    idxs = np.zeros((N_CORES, P, nch * 8), np.int16)
    g = col * P + p_e
    src_reb = (src_s - h_e * HALF).astype(np.int16)
    for rep in range(8):
        idxs[c_e, 16 * rep + g % 16, g // 16] = src_reb
    return k, bases, nch, idxs, dstloc, wcol


def _run_layer(nc, table_pad, prep, wmat):
    _, _, _, idxs, dstloc, wcol = prep
    iota = np.tile(np.arange(P, dtype=np.float32)[None, :], (P, 1)).astype(BF16)
    tlo = table_pad[:HALF]
    thi = table_pad[HALF:]
    ins = []
    for c in range(N_CORES):
        c0 = c * NODES_PER_CORE
        xlocT = np.ascontiguousarray(table_pad[c0:c0 + SLOTS].T)
        ins.append({
            "tlo": tlo, "thi": thi, "idxs": idxs[c], "dstloc": dstloc[c],
            "wcol": wcol[c], "iota": iota, "wmat": wmat, "xlocT": xlocT,
        })
    res = run_bass_kernel_spmd(nc, ins, core_ids=list(range(N_CORES)))
    aggr = np.empty((N, D), np.float32)
    for c in range(N_CORES):
        sl = res.results[c]["out"]  # [128 feat, 6272 pos]
        aggr[c * NODES_PER_CORE:(c + 1) * NODES_PER_CORE] = \
            sl[:, :NODES_PER_CORE].T
    return aggr


def _layernorm(x, g, b):
    mu = x.mean(axis=-1, keepdims=True)
    var = np.square(x - mu).mean(axis=-1, keepdims=True)
    return (x - mu) / np.sqrt(var + EPS_LN) * g + b


def _pad_bf16(x):
    out = np.zeros((N_PAD, D), BF16)
    out[:N] = x.astype(BF16)
    return out


def kernel(user_indices, item_indices, edge_index, edge_type, edge_weight,
           emb, W1_rel, W1_root, b1, g1, be1, W2_rel, W2_root, b2,
           mW1, mb1, mW2, mb2, mW3, mb3, oW, ob):
    user_indices = np.asarray(user_indices)
    item_indices = np.asarray(item_indices)
    edge_index = np.asarray(edge_index)
    edge_type = np.asarray(edge_type)
    edge_weight = np.asarray(edge_weight)
    emb = np.asarray(emb, np.float32)

    prep1 = _prep_edges(edge_index, edge_type, edge_weight)
    needed2 = np.zeros(N, bool)
    needed2[user_indices] = True
    needed2[item_indices] = True
    prep2 = _prep_edges(edge_index, edge_type, edge_weight,
                        mask=needed2[np.asarray(edge_index[1])])
    ncs = []
    for prep in (prep1, prep2):
        k, bases, nch = prep[0], prep[1], prep[2]
        key = tuple(k.ravel())
        if key not in _compiled:
            _compiled[key] = _build_program(k, bases, nch)
        ncs.append(_compiled[key])

    w1 = np.concatenate([np.asarray(W1_rel[0]), np.asarray(W1_rel[1]),
                         np.asarray(W1_root)], axis=1).astype(BF16)
    w2 = np.concatenate([np.asarray(W2_rel[0]), np.asarray(W2_rel[1]),
                         np.asarray(W2_root)], axis=1).astype(BF16)

    aggr1 = _run_layer(ncs[0], _pad_bf16(emb), prep1, w1)
    h = np.maximum(aggr1 + np.asarray(b1)[None, :], 0.0)
    h = _layernorm(h, np.asarray(g1)[None, :], np.asarray(be1)[None, :])

    h2 = _run_layer(ncs[1], _pad_bf16(h), prep2, w2)
    h2 = h2 + np.asarray(b2)[None, :]

    u = h2[user_indices]
    it = h2[item_indices]
    un = u / np.maximum(np.linalg.norm(u, axis=-1, keepdims=True), EPS_NORM)
    itn = it / np.maximum(np.linalg.norm(it, axis=-1, keepdims=True), EPS_NORM)
    gmf = un * itn
    z = np.concatenate([u, it], axis=-1)
    z = np.maximum(z @ np.asarray(mW1) + np.asarray(mb1), 0.0)
    z = np.maximum(z @ np.asarray(mW2) + np.asarray(mb2), 0.0)
    z = np.maximum(z @ np.asarray(mW3) + np.asarray(mb3), 0.0)
    final = np.concatenate([gmf, z], axis=-1)
    score = (final @ np.asarray(oW) + np.asarray(ob)).squeeze(-1)
    return score.astype(np.float32)
